# revision 40
# baseline (speedup 1.0000x reference)
"""Causal self-attention with ALiBi for TRN2, 8 NeuronCores.

Sharding: core c -> batch b = c % 4, head-shard hs = c // 4.
Head-shard hs owns global heads {2j + hs : j in 0..7} (interleaved so both
shards see the same mix of ALiBi slopes -> balanced banded-attention work).

All matmul operands are bf16 (fp32 PSUM accumulate); rel err ~3.5e-3 vs
the fp32 reference, inside the 2e-2 gate with ~5x margin.

Per-core computation (B=1 batch, 8 heads):
  phase 1: QKV projection.  xT [D,S] resident in SBUF (bf16).  Q^T/K^T in
    [col, s] layout (head pairs packed 64+64 into 128-partition tiles, Q
    pre-scaled by 1/sqrt(HD) via host-scaled Wq).  V in [s, col] layout
    with a ones column per head (PV then yields both the unnormalized
    output AND the softmax denominator).
  phase 2: per (head-pair p, q-chunk 512): scores S^T[k,q] = K^T.T @ Q^T
    (K=64 contraction; the two heads run concurrently in separate PE
    row-groups), exp on ACT with per-partition bias slope*(k - qmid)
    (the -slope*q part of ALiBi cancels in softmax; qmid recentering
    prevents overflow; CUT=12 band truncation drops k-tiles whose weights
    are < e^-12 of the kept mass -- invisible next to bf16 noise),
    PV accumulation into PSUM [65, 512] over the k band.
  phase 3: normalize (approx reciprocal of row 64, gpsimd partition
    broadcast), out-projection O^T.T @ Wo accumulated over feature tiles.

Schedule (one fused Tile program, emission order = scheduler priority):
  V -> QK(m3,m7) -> attention p3 (largest band, ~48% of exp work) ->
  QK(m0,m4) -> p0 -> QK(m1,m5) -> p1 -> QK(m2,m6) -> p2 with out-proj
  stripes stitched into p2's stream.  p-major attention starts the ACT
  exp stream mid-projection while later QK m-groups act as PE filler,
  keeping the PE HAM-warm (measured throttle ~7% vs 53% for the
  phase-serial version); the biggest pair goes first and the smallest
  pair + PE-heavy stripes last to minimize the ACT-bound tail.
  A dummy gpsimd partition_broadcast at t~1us preloads the ext-isa
  library that the first normalize would otherwise stall ~8us on.
  V MUST be emitted before any attention group: emitting attention
  first produced nondeterministically corrupted output.  Also learned
  the hard way: DMA cannot read PSUM, and reciprocal_approx_fast
  silently returns garbage on PSUM inputs.

Host side: shard/transpose inputs (bf16), pre-tile weights so every load
is one contiguous [128, N] DMA, run SPMD on 8 cores, sum the two
head-shards' partial outputs per batch, add bo.
"""

import math

import numpy as np

B, S, D, H = 4, 2048, 1024, 16
HD = D // H
NSLOT = 8          # local heads per core
NQC = 4            # q chunks of 512
NKT = 16           # k tiles of 128
SC = 512
KT = 128
NCORES = 8

# ALiBi slopes for global heads
SLOPES = [2.0 ** (-0.5 * (h + 1)) for h in range(H)]

# band cutoff: terms with slope*(q-k) > CUT are < e^-CUT relative to the
# diagonal term; with bf16 attention weights (quantization ~4e-3) a tail
# mass of e^-12/slope / denominator ~ 1e-5 is invisible
CUT = 12.0


def _bt(h):
    """Band width in 128-k-tiles for global head h (delta_max + 1)."""
    d_max = int(math.ceil(CUT / SLOPES[h]))
    return min(NKT, (127 + d_max) // 128 + 1)


def _w(h):
    """Max exp-op width (q columns) for global head h: slope*(W/2) <= 64."""
    s = SLOPES[h]
    if s * 256.0 <= 64.0:
        return 512
    if s * 128.0 <= 64.0:
        return 256
    return 128


# per-slot params = union over the two head shards (program is SPMD-shared)
SLOT_BT = [max(_bt(2 * j), _bt(2 * j + 1)) for j in range(NSLOT)]
SLOT_W = [min(_w(2 * j), _w(2 * j + 1)) for j in range(NSLOT)]


def plan_attention():
    """Enumerate all attention tile ops. Returns (ops, bias_cols) where ops is
    a list of dicts and bias_cols maps (slot, mkey) -> expb column index."""
    bias_cols = {}
    ops = []
    for qc in range(NQC):
        for p in range(4):
            for kt in range(4 * qc + 4):
                for half in (0, 1):
                    j = 2 * p + half
                    bt, w = SLOT_BT[j], SLOT_W[j]
                    lo = max(0, 4 * qc - bt + 1)
                    if kt < lo:
                        continue
                    qs_start = max(4 * qc, kt)
                    qs_end = min(4 * qc + 3, kt + bt - 1)
                    if qs_start > qs_end:
                        continue
                    c0 = 128 * (qs_start - 4 * qc)
                    c1 = 128 * (qs_end - 4 * qc) + 128
                    # exp ops aligned to an absolute w-grid within the qc
                    # chunk: qmid (the recentering constant) must depend only
                    # on the column block, never on kt, so that every term
                    # entering a given column's softmax sum carries the same
                    # exp(-slope*qmid) factor.
                    exps = []
                    for g in range((c0 // w) * w, c1, w):
                        a, e = max(c0, g), min(c1, g + w)
                        if a >= e:
                            continue
                        mkey = (512 * qc + g + w // 2) - 128 * kt
                        col = bias_cols.setdefault((j, mkey), len(bias_cols))
                        exps.append((a, e - a, col))
                    ops.append(dict(qc=qc, p=p, half=half, j=j, kt=kt,
                                    c0=c0, c1=c1, exps=exps,
                                    tril=(kt >= 4 * qc),
                                    first=(kt == lo), last=(kt == 4 * qc + 3)))
    return ops, bias_cols


ATT_OPS, BIAS_COLS = plan_attention()
NBIAS = len(BIAS_COLS)

_nc_cache = None


def build_program():
    global _nc_cache
    if _nc_cache is not None:
        return _nc_cache

    import concourse.bacc as bacc
    import concourse.tile as tile
    from concourse import mybir

    F32 = mybir.dt.float32
    F32R = mybir.dt.float32r
    BF16 = mybir.dt.bfloat16
    EXP = mybir.ActivationFunctionType.Exp
    COPY = mybir.ActivationFunctionType.Copy

    nc = bacc.Bacc("TRN2", target_bir_lowering=False, debug=False,
                   num_devices=NCORES)

    xT_d = nc.dram_tensor("xT", [D, S], BF16, kind="ExternalInput")
    # QK weights pre-tiled host-side: [p, m(8), d(8), c(128)] -> [128, 8192]
    wqkvT_d = nc.dram_tensor("wqkvT", [128, 8192], BF16, kind="ExternalInput")
    # V weights pre-tiled host-side: [p, d(8), c(512)] -> [128, 4096]
    wvT_d = nc.dram_tensor("wvT", [128, 4096], BF16, kind="ExternalInput")
    wo_d = nc.dram_tensor("wo", [512, D], BF16, kind="ExternalInput")
    qkb_d = nc.dram_tensor("qkb", [128, 8], F32, kind="ExternalInput")
    bvr_d = nc.dram_tensor("bvr", [128, 512], F32, kind="ExternalInput")
    expb_d = nc.dram_tensor("expb", [128, max(NBIAS, 1)], F32,
                            kind="ExternalInput")
    tril_d = nc.dram_tensor("tril", [128, 128], BF16, kind="ExternalInput")
    ones_d = nc.dram_tensor("ones64", [1, 64], F32, kind="ExternalInput")
    vones_d = nc.dram_tensor("vones", [128, 8], BF16, kind="ExternalInput")
    out_d = nc.dram_tensor("out_p", [S, D], F32, kind="ExternalOutput")

    with tile.TileContext(nc) as tc:
        with nc.allow_low_precision(reason="bf16 attention kernel"), \
             tc.tile_pool(name="persist", bufs=1) as pp, \
             tc.tile_pool(name="expsp", bufs=8) as expsp, \
             tc.tile_pool(name="p1sb", bufs=1) as p1sb, \
             tc.tile_pool(name="p2sb", bufs=1) as p2sb, \
             tc.tile_pool(name="rcp", bufs=4) as rcp, \
             tc.tile_pool(name="rbp", bufs=4) as rbp, \
             tc.tile_pool(name="outp", bufs=4) as outp, \
             tc.tile_pool(name="qkps", bufs=2, space="PSUM") as qkps, \
             tc.tile_pool(name="sps", bufs=3, space="PSUM") as sps, \
             tc.tile_pool(name="ops_", bufs=3, space="PSUM") as ops_:

            # ---- persistent tiles ----
            qkT_Q = [pp.tile([128, S], BF16, name=f"qkTQ{p}") for p in range(4)]
            qkT_K = [pp.tile([128, S], BF16, name=f"qkTK{p}") for p in range(4)]
            Vbuf = [pp.tile([128, NSLOT * 65], BF16, name=f"vb{t}")
                    for t in range(NKT)]
            qkb_t = pp.tile([128, 8], F32, name="qkb_t")
            bvr_t = pp.tile([128, 512], F32, name="bvr_t")
            expb_t = pp.tile([128, max(NBIAS, 1)], F32, name="expb_t")
            tril_t = pp.tile([128, 128], BF16, name="tril_t")
            vones_t = pp.tile([128, 8], BF16, name="vones_t")

            # consts on gpsimd (tiny; done before the warmup broadcast's
            # library load occupies the engine)
            nc.gpsimd.dma_start(out=qkb_t, in_=qkb_d[:, :])
            nc.gpsimd.dma_start(out=bvr_t, in_=bvr_d[:, :])
            nc.gpsimd.dma_start(out=expb_t, in_=expb_d[:, :])
            nc.gpsimd.dma_start(out=tril_t, in_=tril_d[:, :])
            nc.gpsimd.dma_start(out=vones_t, in_=vones_d[:, :])

            # dummy partition_broadcast so the gpsimd ext-isa library loads
            # (~8us ModifyPoolConfig + IRAM DMA) at t~1us instead of stalling
            # the first normalize
            warm_t = pp.tile([64, 8], F32, name="warm_t")
            nc.gpsimd.partition_broadcast(warm_t, qkb_t[0:1, :])

            # ones columns of Vbuf (col 64 of each 65-wide head group)
            for t in range(NKT):
                ones_view = Vbuf[t].rearrange("p (h c) -> p h c", c=65)[:, :, 64:65]
                nc.vector.tensor_copy(ones_view, vones_t.unsqueeze(2))

            # ---- phase-1 inputs ----
            # large DMAs only: small (<1MiB) transfers measured ~60-120 GB/s
            # effective.  xT comes as four 1MiB column-quarters (512 s-cols x
            # all 8 d-tiles) with separate completions, aligned to the V
            # s-tiles and QK s-chunks, so V starts on quarter 0 at ~10us
            # while the rest streams in.
            xTq = [p1sb.tile([128, 8 * 512], BF16, name=f"xTq{q}")
                   for q in range(4)]
            for q in range(4):
                nc.sync.dma_start(
                    out=xTq[q].rearrange("p (d c) -> p d c", c=512),
                    in_=xT_d.rearrange("(d p) c -> p d c",
                                       p=128)[:, :, 512 * q:512 * (q + 1)])

            def xs(d, c0, c1):
                """view of xT[d][:, c0:c1]; [c0, c1) must stay inside one
                512-col quarter."""
                q = c0 // 512
                assert c1 <= 512 * (q + 1)
                return xTq[q][:, 512 * d + c0 - 512 * q:
                              512 * d + c1 - 512 * q]
            wvall = p1sb.tile([128, 4096], BF16, name="wvall")
            wv = [wvall[:, 512 * d:512 * (d + 1)] for d in range(8)]
            nc.scalar.dma_start(out=wvall, in_=wvT_d[:, :])
            wqall = p1sb.tile([128, 8192], BF16, name="wqall")
            nc.scalar.dma_start(out=wqall, in_=wqkvT_d[:, :])

            OT = [p2sb.tile([128, S], BF16, name=f"OT{p}") for p in range(4)]
            wo_t = [p2sb.tile([128, D], BF16, name=f"wo{d}") for d in range(4)]
            for d in range(4):
                nc.gpsimd.dma_start(
                    out=wo_t[d], in_=wo_d[128 * d:128 * (d + 1), :])

            # ---- emission helpers ----
            def emit_qk(m):
                """project m-tile m (Q cols if m<4 else K cols)."""
                wts = [wqall[:, 1024 * m + 128 * d:1024 * m + 128 * (d + 1)]
                       for d in range(8)]
                dst = qkT_Q[m] if m < 4 else qkT_K[m - 4]
                for sh in range(2):
                    psq = [qkps.tile([128, SC], F32, name="psq", tag="ps1")
                           for _ in range(2)]
                    for d in range(8):
                        for si in range(2):
                            s = 2 * sh + si
                            nc.tensor.matmul(
                                psq[si], wts[d],
                                xs(d, SC * s, SC * (s + 1)),
                                start=(d == 0), stop=(d == 7))
                    for si in range(2):
                        s = 2 * sh + si
                        nc.vector.tensor_scalar_add(
                            dst[:, SC * s:SC * (s + 1)], psq[si],
                            qkb_t[:, m:m + 1])

            def emit_v():
                """V projection: [s, col] layout, 16 s-tiles."""
                for st in range(NKT):
                    psv = qkps.tile([128, 512], F32, name="psv", tag="ps1")
                    for d in range(8):
                        nc.tensor.matmul(
                            psv, xs(d, 128 * st, 128 * (st + 1)), wv[d],
                            start=(d == 0), stop=(d == 7))
                    vdst = Vbuf[st].rearrange("p (h c) -> p h c", c=65)[:, :, 0:64]
                    nc.vector.tensor_tensor(
                        vdst, psv.rearrange("p (g c) -> p g c", c=64),
                        bvr_t.rearrange("p (g c) -> p g c", c=64),
                        op=mybir.AluOpType.add)

            # attention plan grouped per (p, qc) for p-major emission
            plan = {}
            for o in ATT_OPS:
                plan.setdefault((o["p"], o["qc"]), []).append(o)

            def emit_att(p, qc):
                """scores -> exp -> PV -> normalize for one (p, qc) group."""
                psumO = {}
                for half in (0, 1):
                    psumO[half] = ops_.tile([65, SC], F32, name="psumO")
                groups = []
                for o in plan[(p, qc)]:
                    if groups and groups[-1][0]["kt"] == o["kt"]:
                        groups[-1].append(o)
                    else:
                        groups.append([o])

                # software pipeline: PV(kt) is emitted after scores(kt+PIPE)
                PIPE = 2
                pend = []

                def emit_scores(grp):
                    out = []
                    for o in grp:
                        half, kt = o["half"], o["kt"]
                        c0, c1 = o["c0"], o["c1"]
                        rb0 = 64 * half
                        psS = sps.tile([128, SC], F32, name="psS")
                        nc.tensor.matmul(
                            psS[:, c0:c1],
                            qkT_K[p][rb0:rb0 + 64, 128 * kt:128 * (kt + 1)],
                            qkT_Q[p][rb0:rb0 + 64,
                                     SC * qc + c0:SC * qc + c1],
                            start=True, stop=True)
                        eS = expsp.tile([128, SC], BF16, name="eS")
                        for (a, ww, col) in o["exps"]:
                            nc.scalar.activation(
                                eS[:, a:a + ww], psS[:, a:a + ww], EXP,
                                bias=expb_t[:, col:col + 1], scale=1.0)
                        if o["tril"]:
                            nc.vector.tensor_mul(
                                eS[:, c0:c0 + 128], eS[:, c0:c0 + 128],
                                tril_t)
                        out.append((o, eS))
                    return out

                def emit_pv(ready):
                    for (o, eS) in ready:
                        c0, c1 = o["c0"], o["c1"]
                        nc.tensor.matmul(
                            psumO[o["half"]][0:65, c0:c1],
                            Vbuf[o["kt"]][:, 65 * o["j"]:65 * o["j"] + 65],
                            eS[:, c0:c1],
                            start=o["first"], stop=o["last"])

                for grp in groups:
                    pend.append(emit_scores(grp))
                    if len(pend) > PIPE:
                        emit_pv(pend.pop(0))
                for ready in pend:
                    emit_pv(ready)

                # normalize both halves (no PE involvement)
                for half in (0, 1):
                    ssum = rcp.tile([1, SC], F32, name="ssum")
                    nc.vector.tensor_copy(ssum, psumO[half][64:65, :])
                    rc = rcp.tile([1, SC], F32, name="rc")
                    nc.vector.reciprocal_approx_fast(rc, ssum)
                    rb = rbp.tile([64, SC], F32, name="rb")
                    nc.gpsimd.partition_broadcast(rb, rc)
                    nc.vector.tensor_mul(
                        OT[p][64 * half:64 * half + 64,
                              SC * qc:SC * (qc + 1)],
                        psumO[half][0:64, :],
                        rb)

            def emit_stripe(qc):
                """out-projection stripe for q-chunk qc (needs all p)."""
                for st in range(4 * qc, 4 * qc + 4):
                    pse = [qkps.tile([128, SC], F32, name="pse", tag="ps1")
                           for _ in range(2)]
                    for d in range(4):
                        for e in range(2):
                            nc.tensor.matmul(
                                pse[e],
                                OT[d][:, 128 * st:128 * (st + 1)],
                                wo_t[d][:, SC * e:SC * (e + 1)],
                                start=(d == 0), stop=(d == 3))
                    for e in range(2):
                        ob = outp.tile([128, SC], F32, name="ob")
                        nc.scalar.copy(ob, pse[e])
                        nc.gpsimd.dma_start(
                            out=out_d[128 * st:128 * (st + 1),
                                      SC * e:SC * (e + 1)],
                            in_=ob)

            # ---- global emission order (p-major attention) ----
            # V first (so PV never outruns Vbuf), then QK for p0 so exp work
            # starts ~40us in; later QK m-groups act as PE filler while ACT
            # drains each p's exps; out-proj stripes are stitched into p3's
            # stream as tail filler.
            emit_v()
            emit_qk(3)
            emit_qk(7)
            for qc in range(NQC):
                emit_att(3, qc)
            emit_qk(0)
            emit_qk(4)
            for qc in range(NQC):
                emit_att(0, qc)
            emit_qk(1)
            emit_qk(5)
            for qc in range(NQC):
                emit_att(1, qc)
            emit_qk(2)
            emit_qk(6)
            for qc in range(NQC):
                emit_att(2, qc)
                emit_stripe(qc)

    nc.compile()
    _nc_cache = nc
    return nc


def make_inputs(x, mask, Wqkv, bqkv, Wo, bo):
    """Build the 8 per-core input maps."""
    x = np.ascontiguousarray(x, dtype=np.float32)
    Wqkv = np.asarray(Wqkv, dtype=np.float32)
    bqkv = np.asarray(bqkv, dtype=np.float32)
    Wo = np.asarray(Wo, dtype=np.float32)

    import ml_dtypes
    bf = ml_dtypes.bfloat16

    # diagonal-block mask in [k_partition, q_column] layout: keep k <= q,
    # i.e. partition p <= column c -> UPPER-triangular
    tril = np.triu(np.ones((128, 128), dtype=np.float32)).astype(bf)
    ones64 = np.ones((1, 64), dtype=np.float32)
    vones = np.ones((128, 8), dtype=bf)
    p_idx = np.arange(128, dtype=np.float32)[:, None]

    in_maps = []
    for c in range(NCORES):
        b, hs = c % 4, c // 4
        heads = [2 * j + hs for j in range(NSLOT)]
        # column order: Q cols (slot-major), K cols, V cols
        qcols = np.concatenate(
            [np.arange(0 * D + h * HD, 0 * D + h * HD + HD) for h in heads])
        kcols = np.concatenate(
            [np.arange(1 * D + h * HD, 1 * D + h * HD + HD) for h in heads])
        vcols = np.concatenate(
            [np.arange(2 * D + h * HD, 2 * D + h * HD + HD) for h in heads])
        cols = np.concatenate([qcols, kcols, vcols])
        wqkv = Wqkv[:, cols].copy()
        bq = bqkv[cols].copy()
        wqkv[:, :512] *= 0.125  # fold 1/sqrt(HD) into Q
        bq[:512] *= 0.125

        qkb = bq[:1024].reshape(8, 128).T.copy()       # [128, m-tile]
        bvr = np.broadcast_to(bq[1024:], (128, 512)).copy()

        expb = np.zeros((128, max(NBIAS, 1)), dtype=np.float32)
        for (j, mkey), col in BIAS_COLS.items():
            expb[:, col:col + 1] = SLOPES[2 * j + hs] * (p_idx - mkey)

        rows = np.concatenate(
            [np.arange(h * HD, h * HD + HD) for h in heads])
        wo = Wo[rows, :].copy()

        # pre-tile QK/V weights to match the SBUF tile layout so each load
        # is one contiguous [128, N] DMA:
        #   wqkvT[p, m, d, c] = wqkv[128d+p, 128m+c]   (m-major, 8x8x128)
        #   wvT[p, d, c]      = wqkv[128d+p, 1024+c]
        wqk = wqkv[:, :1024].reshape(8, 128, 8, 128)
        wqkvT = np.ascontiguousarray(
            wqk.transpose(1, 2, 0, 3).reshape(128, 8192)).astype(bf)
        wvp = wqkv[:, 1024:].reshape(8, 128, 512)
        wvT = np.ascontiguousarray(
            wvp.transpose(1, 0, 2).reshape(128, 4096)).astype(bf)

        in_maps.append({
            "xT": np.ascontiguousarray(x[b].T).astype(bf),
            "wqkvT": wqkvT,
            "wvT": wvT,
            "wo": np.ascontiguousarray(wo).astype(bf),
            "qkb": np.ascontiguousarray(qkb),
            "bvr": bvr,
            "expb": expb,
            "tril": tril,
            "ones64": ones64,
            "vones": vones,
        })
    return in_maps


def kernel(x, mask, Wqkv, bqkv, Wo, bo, _trace=False):
    from concourse.bass_utils import run_bass_kernel_spmd

    nc = build_program()
    in_maps = make_inputs(x, mask, Wqkv, bqkv, Wo, bo)
    res = run_bass_kernel_spmd(nc, in_maps, core_ids=list(range(NCORES)),
                               trace=_trace, trace_cores=[0] if _trace else None)
    bo = np.asarray(bo, dtype=np.float32)
    out = np.empty((B, S, D), dtype=np.float32)
    for b in range(B):
        out[b] = res.results[b]["out_p"] + res.results[b + 4]["out_p"] + bo
    if _trace:
        kernel._last_result = res
    return out



# revision 42
# speedup vs baseline: 1.1972x; 1.1972x over previous
"""Causal self-attention with ALiBi for TRN2, 8 NeuronCores.

Sharding: core c -> batch b = c % 4, head-shard hs = c // 4.
Head-shard hs owns global heads {2j + hs : j in 0..7} (interleaved so both
shards see the same mix of ALiBi slopes -> balanced banded-attention work).

All matmul operands are bf16 (fp32 PSUM accumulate); rel err ~3.5e-3 vs
the fp32 reference, inside the 2e-2 gate with ~5x margin.

Per-core computation (B=1 batch, 8 heads):
  phase 1: QKV projection.  xT [D,S] resident in SBUF (bf16).  Q^T/K^T in
    [col, s] layout (head pairs packed 64+64 into 128-partition tiles, Q
    pre-scaled by 1/sqrt(HD) via host-scaled Wq).  V in [s, col] layout
    with a ones column per head (PV then yields both the unnormalized
    output AND the softmax denominator).
  phase 2: per (head-pair p, q-chunk 512): scores S^T[k,q] = K^T.T @ Q^T
    (K=64 contraction; the two heads run concurrently in separate PE
    row-groups), exp on ACT with per-partition bias slope*(k - qmid)
    (the -slope*q part of ALiBi cancels in softmax; qmid recentering
    prevents overflow; CUT=12 band truncation drops k-tiles whose weights
    are < e^-12 of the kept mass -- invisible next to bf16 noise),
    PV accumulation into PSUM [65, 512] over the k band.
  phase 3: normalize (approx reciprocal of row 64, gpsimd partition
    broadcast), out-projection O^T.T @ Wo accumulated over feature tiles.

Schedule (one fused Tile program, emission order = scheduler priority):
  V -> QK(m3,m7) -> attention p3 (largest band, ~48% of exp work) ->
  QK(m0,m4) -> p0 -> QK(m1,m5) -> p1 -> QK(m2,m6) -> p2 with out-proj
  stripes stitched into p2's stream.  p-major attention starts the ACT
  exp stream mid-projection while later QK m-groups act as PE filler,
  keeping the PE HAM-warm (measured throttle ~7% vs 53% for the
  phase-serial version); the biggest pair goes first and the smallest
  pair + PE-heavy stripes last to minimize the ACT-bound tail.
  A dummy gpsimd partition_broadcast at t~1us preloads the ext-isa
  library that the first normalize would otherwise stall ~8us on.
  V MUST be emitted before any attention group: emitting attention
  first produced nondeterministically corrupted output.  Also learned
  the hard way: DMA cannot read PSUM, and reciprocal_approx_fast
  silently returns garbage on PSUM inputs.

Host side: shard/transpose inputs (bf16), pre-tile weights so every load
is one contiguous [128, N] DMA, run SPMD on 8 cores, sum the two
head-shards' partial outputs per batch, add bo.
"""

import math

import numpy as np

B, S, D, H = 4, 2048, 1024, 16
HD = D // H
NSLOT = 8          # local heads per core
NQC = 4            # q chunks of 512
NKT = 16           # k tiles of 128
SC = 512
KT = 128
NCORES = 8

# ALiBi slopes for global heads
SLOPES = [2.0 ** (-0.5 * (h + 1)) for h in range(H)]

# band cutoff: terms with slope*(q-k) > CUT are < e^-CUT relative to the
# diagonal term; with bf16 attention weights (quantization ~4e-3) a tail
# mass of e^-12/slope / denominator ~ 1e-5 is invisible
CUT = 12.0


def _bt(h):
    """Band width in 128-k-tiles for global head h (delta_max + 1)."""
    d_max = int(math.ceil(CUT / SLOPES[h]))
    return min(NKT, (127 + d_max) // 128 + 1)


def _w(h):
    """Max exp-op width (q columns) for global head h: slope*(W/2) <= 64."""
    s = SLOPES[h]
    if s * 256.0 <= 64.0:
        return 512
    if s * 128.0 <= 64.0:
        return 256
    return 128


# per-slot params = union over the two head shards (program is SPMD-shared)
SLOT_BT = [max(_bt(2 * j), _bt(2 * j + 1)) for j in range(NSLOT)]
SLOT_W = [min(_w(2 * j), _w(2 * j + 1)) for j in range(NSLOT)]


def plan_attention():
    """Enumerate all attention tile ops. Returns (ops, bias_cols) where ops is
    a list of dicts and bias_cols maps (slot, mkey) -> expb column index."""
    bias_cols = {}
    ops = []
    for qc in range(NQC):
        for p in range(4):
            for kt in range(4 * qc + 4):
                for half in (0, 1):
                    j = 2 * p + half
                    bt, w = SLOT_BT[j], SLOT_W[j]
                    lo = max(0, 4 * qc - bt + 1)
                    if kt < lo:
                        continue
                    qs_start = max(4 * qc, kt)
                    qs_end = min(4 * qc + 3, kt + bt - 1)
                    if qs_start > qs_end:
                        continue
                    c0 = 128 * (qs_start - 4 * qc)
                    c1 = 128 * (qs_end - 4 * qc) + 128
                    # exp ops aligned to an absolute w-grid within the qc
                    # chunk: qmid (the recentering constant) must depend only
                    # on the column block, never on kt, so that every term
                    # entering a given column's softmax sum carries the same
                    # exp(-slope*qmid) factor.
                    exps = []
                    for g in range((c0 // w) * w, c1, w):
                        a, e = max(c0, g), min(c1, g + w)
                        if a >= e:
                            continue
                        mkey = (512 * qc + g + w // 2) - 128 * kt
                        col = bias_cols.setdefault((j, mkey), len(bias_cols))
                        exps.append((a, e - a, col))
                    ops.append(dict(qc=qc, p=p, half=half, j=j, kt=kt,
                                    c0=c0, c1=c1, exps=exps,
                                    tril=(kt >= 4 * qc),
                                    first=(kt == lo), last=(kt == 4 * qc + 3)))
    return ops, bias_cols


ATT_OPS, BIAS_COLS = plan_attention()
NBIAS = len(BIAS_COLS)

_nc_cache = None


def build_program():
    global _nc_cache
    if _nc_cache is not None:
        return _nc_cache

    import concourse.bacc as bacc
    import concourse.tile as tile
    from concourse import mybir

    F32 = mybir.dt.float32
    F32R = mybir.dt.float32r
    BF16 = mybir.dt.bfloat16
    EXP = mybir.ActivationFunctionType.Exp
    COPY = mybir.ActivationFunctionType.Copy

    nc = bacc.Bacc("TRN2", target_bir_lowering=False, debug=False,
                   num_devices=NCORES)

    # x pre-tiled host-side into 4 column-quarters, each contiguous:
    # xTq[p, q, d, c] = x.T[128d+p, 512q+c]  ->  [128, 16384]
    xTq_d = nc.dram_tensor("xTq", [128, 16384], BF16, kind="ExternalInput")
    # QK weights pre-tiled host-side: [p, m(8), d(8), c(128)] -> [128, 8192]
    wqkvT_d = nc.dram_tensor("wqkvT", [128, 8192], BF16, kind="ExternalInput")
    # V weights pre-tiled host-side: [p, d(8), c(512)] -> [128, 4096]
    wvT_d = nc.dram_tensor("wvT", [128, 4096], BF16, kind="ExternalInput")
    wo_d = nc.dram_tensor("wo", [512, D], BF16, kind="ExternalInput")
    qkb_d = nc.dram_tensor("qkb", [128, 8], F32, kind="ExternalInput")
    bvr_d = nc.dram_tensor("bvr", [128, 512], F32, kind="ExternalInput")
    expb_d = nc.dram_tensor("expb", [128, max(NBIAS, 1)], F32,
                            kind="ExternalInput")
    tril_d = nc.dram_tensor("tril", [128, 128], BF16, kind="ExternalInput")
    ones_d = nc.dram_tensor("ones64", [1, 64], F32, kind="ExternalInput")
    vones_d = nc.dram_tensor("vones", [128, 8], BF16, kind="ExternalInput")
    out_d = nc.dram_tensor("out_p", [S, D], F32, kind="ExternalOutput")

    with tile.TileContext(nc) as tc:
        with nc.allow_low_precision(reason="bf16 attention kernel"), \
             tc.tile_pool(name="persist", bufs=1) as pp, \
             tc.tile_pool(name="expsp", bufs=8) as expsp, \
             tc.tile_pool(name="p1sb", bufs=1) as p1sb, \
             tc.tile_pool(name="p2sb", bufs=1) as p2sb, \
             tc.tile_pool(name="rcp", bufs=4) as rcp, \
             tc.tile_pool(name="rbp", bufs=4) as rbp, \
             tc.tile_pool(name="outp", bufs=4) as outp, \
             tc.tile_pool(name="qkps", bufs=2, space="PSUM") as qkps, \
             tc.tile_pool(name="sps", bufs=3, space="PSUM") as sps, \
             tc.tile_pool(name="ops_", bufs=3, space="PSUM") as ops_:

            # ---- persistent tiles ----
            qkT_Q = [pp.tile([128, S], BF16, name=f"qkTQ{p}") for p in range(4)]
            qkT_K = [pp.tile([128, S], BF16, name=f"qkTK{p}") for p in range(4)]
            Vbuf = [pp.tile([128, NSLOT * 65], BF16, name=f"vb{t}")
                    for t in range(NKT)]
            qkb_t = pp.tile([128, 8], F32, name="qkb_t")
            bvr_t = pp.tile([128, 512], F32, name="bvr_t")
            expb_t = pp.tile([128, max(NBIAS, 1)], F32, name="expb_t")
            tril_t = pp.tile([128, 128], BF16, name="tril_t")
            vones_t = pp.tile([128, 8], BF16, name="vones_t")

            # consts on gpsimd (tiny; done before the warmup broadcast's
            # library load occupies the engine)
            nc.gpsimd.dma_start(out=qkb_t, in_=qkb_d[:, :])
            nc.gpsimd.dma_start(out=bvr_t, in_=bvr_d[:, :])
            nc.gpsimd.dma_start(out=expb_t, in_=expb_d[:, :])
            nc.gpsimd.dma_start(out=tril_t, in_=tril_d[:, :])
            nc.gpsimd.dma_start(out=vones_t, in_=vones_d[:, :])

            # dummy partition_broadcast so the gpsimd ext-isa library loads
            # (~8us ModifyPoolConfig + IRAM DMA) at t~1us instead of stalling
            # the first normalize
            warm_t = pp.tile([64, 8], F32, name="warm_t")
            nc.gpsimd.partition_broadcast(warm_t, qkb_t[0:1, :])

            # ones columns of Vbuf (col 64 of each 65-wide head group)
            for t in range(NKT):
                ones_view = Vbuf[t].rearrange("p (h c) -> p h c", c=65)[:, :, 64:65]
                nc.vector.tensor_copy(ones_view, vones_t.unsqueeze(2))

            # ---- phase-1 inputs ----
            # one large DMA per tensor: small (<1MiB) transfers measured only
            # ~60-120 GB/s effective (per-transfer completion latency), which
            # paced the whole projection phase
            # four contiguous 1MiB quarter transfers with separate
            # completions, aligned to the V s-tiles / QK s-chunks: V starts
            # on quarter 0 at ~10us while the rest streams in
            xTq = [p1sb.tile([128, 4096], BF16, name=f"xTq{q}")
                   for q in range(4)]
            for q in range(4):
                nc.sync.dma_start(
                    out=xTq[q], in_=xTq_d[:, 4096 * q:4096 * (q + 1)])

            def xs(d, c0, c1):
                """view of x.T[128d:128d+128, c0:c1]; [c0, c1) must stay
                inside one 512-col quarter."""
                q = c0 // 512
                assert c1 <= 512 * (q + 1)
                return xTq[q][:, 512 * d + c0 - 512 * q:
                              512 * d + c1 - 512 * q]
            wvall = p1sb.tile([128, 4096], BF16, name="wvall")
            wv = [wvall[:, 512 * d:512 * (d + 1)] for d in range(8)]
            nc.scalar.dma_start(out=wvall, in_=wvT_d[:, :])
            wqall = p1sb.tile([128, 8192], BF16, name="wqall")
            nc.scalar.dma_start(out=wqall, in_=wqkvT_d[:, :])

            OT = [p2sb.tile([128, S], BF16, name=f"OT{p}") for p in range(4)]
            wo_t = [p2sb.tile([128, D], BF16, name=f"wo{d}") for d in range(4)]
            for d in range(4):
                nc.gpsimd.dma_start(
                    out=wo_t[d], in_=wo_d[128 * d:128 * (d + 1), :])

            # ---- emission helpers ----
            def emit_qk(m):
                """project m-tile m (Q cols if m<4 else K cols)."""
                wts = [wqall[:, 1024 * m + 128 * d:1024 * m + 128 * (d + 1)]
                       for d in range(8)]
                dst = qkT_Q[m] if m < 4 else qkT_K[m - 4]
                for sh in range(2):
                    psq = [qkps.tile([128, SC], F32, name="psq", tag="ps1")
                           for _ in range(2)]
                    for d in range(8):
                        for si in range(2):
                            s = 2 * sh + si
                            nc.tensor.matmul(
                                psq[si], wts[d],
                                xs(d, SC * s, SC * (s + 1)),
                                start=(d == 0), stop=(d == 7))
                    for si in range(2):
                        s = 2 * sh + si
                        nc.vector.tensor_scalar_add(
                            dst[:, SC * s:SC * (s + 1)], psq[si],
                            qkb_t[:, m:m + 1])

            def emit_v():
                """V projection: [s, col] layout, 16 s-tiles."""
                for st in range(NKT):
                    psv = qkps.tile([128, 512], F32, name="psv", tag="ps1")
                    for d in range(8):
                        nc.tensor.matmul(
                            psv, xs(d, 128 * st, 128 * (st + 1)), wv[d],
                            start=(d == 0), stop=(d == 7))
                    vdst = Vbuf[st].rearrange("p (h c) -> p h c", c=65)[:, :, 0:64]
                    nc.vector.tensor_tensor(
                        vdst, psv.rearrange("p (g c) -> p g c", c=64),
                        bvr_t.rearrange("p (g c) -> p g c", c=64),
                        op=mybir.AluOpType.add)

            # attention plan grouped per (p, qc) for p-major emission
            plan = {}
            for o in ATT_OPS:
                plan.setdefault((o["p"], o["qc"]), []).append(o)

            def emit_att(p, qc):
                """scores -> exp -> PV -> normalize for one (p, qc) group."""
                psumO = {}
                for half in (0, 1):
                    psumO[half] = ops_.tile([65, SC], F32, name="psumO")
                groups = []
                for o in plan[(p, qc)]:
                    if groups and groups[-1][0]["kt"] == o["kt"]:
                        groups[-1].append(o)
                    else:
                        groups.append([o])

                # software pipeline: PV(kt) is emitted after scores(kt+PIPE)
                PIPE = 2
                pend = []

                def emit_scores(grp):
                    out = []
                    for o in grp:
                        half, kt = o["half"], o["kt"]
                        c0, c1 = o["c0"], o["c1"]
                        rb0 = 64 * half
                        psS = sps.tile([128, SC], F32, name="psS")
                        nc.tensor.matmul(
                            psS[:, c0:c1],
                            qkT_K[p][rb0:rb0 + 64, 128 * kt:128 * (kt + 1)],
                            qkT_Q[p][rb0:rb0 + 64,
                                     SC * qc + c0:SC * qc + c1],
                            start=True, stop=True)
                        eS = expsp.tile([128, SC], BF16, name="eS")
                        for (a, ww, col) in o["exps"]:
                            nc.scalar.activation(
                                eS[:, a:a + ww], psS[:, a:a + ww], EXP,
                                bias=expb_t[:, col:col + 1], scale=1.0)
                        if o["tril"]:
                            nc.vector.tensor_mul(
                                eS[:, c0:c0 + 128], eS[:, c0:c0 + 128],
                                tril_t)
                        out.append((o, eS))
                    return out

                def emit_pv(ready):
                    for (o, eS) in ready:
                        c0, c1 = o["c0"], o["c1"]
                        nc.tensor.matmul(
                            psumO[o["half"]][0:65, c0:c1],
                            Vbuf[o["kt"]][:, 65 * o["j"]:65 * o["j"] + 65],
                            eS[:, c0:c1],
                            start=o["first"], stop=o["last"])

                for grp in groups:
                    pend.append(emit_scores(grp))
                    if len(pend) > PIPE:
                        emit_pv(pend.pop(0))
                for ready in pend:
                    emit_pv(ready)

                # normalize both halves (no PE involvement)
                for half in (0, 1):
                    ssum = rcp.tile([1, SC], F32, name="ssum")
                    nc.vector.tensor_copy(ssum, psumO[half][64:65, :])
                    rc = rcp.tile([1, SC], F32, name="rc")
                    nc.vector.reciprocal_approx_fast(rc, ssum)
                    rb = rbp.tile([64, SC], F32, name="rb")
                    nc.gpsimd.partition_broadcast(rb, rc)
                    nc.vector.tensor_mul(
                        OT[p][64 * half:64 * half + 64,
                              SC * qc:SC * (qc + 1)],
                        psumO[half][0:64, :],
                        rb)

            def emit_stripe(qc):
                """out-projection stripe for q-chunk qc (needs all p)."""
                for st in range(4 * qc, 4 * qc + 4):
                    pse = [qkps.tile([128, SC], F32, name="pse", tag="ps1")
                           for _ in range(2)]
                    for d in range(4):
                        for e in range(2):
                            nc.tensor.matmul(
                                pse[e],
                                OT[d][:, 128 * st:128 * (st + 1)],
                                wo_t[d][:, SC * e:SC * (e + 1)],
                                start=(d == 0), stop=(d == 3))
                    for e in range(2):
                        ob = outp.tile([128, SC], F32, name="ob")
                        nc.scalar.copy(ob, pse[e])
                        nc.gpsimd.dma_start(
                            out=out_d[128 * st:128 * (st + 1),
                                      SC * e:SC * (e + 1)],
                            in_=ob)

            # ---- global emission order (p-major attention) ----
            # V first (so PV never outruns Vbuf), then QK for p0 so exp work
            # starts ~40us in; later QK m-groups act as PE filler while ACT
            # drains each p's exps; out-proj stripes are stitched into p3's
            # stream as tail filler.
            emit_v()
            emit_qk(3)
            emit_qk(7)
            for qc in range(NQC):
                emit_att(3, qc)
            emit_qk(0)
            emit_qk(4)
            for qc in range(NQC):
                emit_att(0, qc)
            emit_qk(1)
            emit_qk(5)
            for qc in range(NQC):
                emit_att(1, qc)
            emit_qk(2)
            emit_qk(6)
            for qc in range(NQC):
                emit_att(2, qc)
                emit_stripe(qc)

    nc.compile()
    _nc_cache = nc
    return nc


def make_inputs(x, mask, Wqkv, bqkv, Wo, bo):
    """Build the 8 per-core input maps."""
    x = np.ascontiguousarray(x, dtype=np.float32)
    Wqkv = np.asarray(Wqkv, dtype=np.float32)
    bqkv = np.asarray(bqkv, dtype=np.float32)
    Wo = np.asarray(Wo, dtype=np.float32)

    import ml_dtypes
    bf = ml_dtypes.bfloat16

    # diagonal-block mask in [k_partition, q_column] layout: keep k <= q,
    # i.e. partition p <= column c -> UPPER-triangular
    tril = np.triu(np.ones((128, 128), dtype=np.float32)).astype(bf)
    ones64 = np.ones((1, 64), dtype=np.float32)
    vones = np.ones((128, 8), dtype=bf)
    p_idx = np.arange(128, dtype=np.float32)[:, None]

    in_maps = []
    for c in range(NCORES):
        b, hs = c % 4, c // 4
        heads = [2 * j + hs for j in range(NSLOT)]
        # column order: Q cols (slot-major), K cols, V cols
        qcols = np.concatenate(
            [np.arange(0 * D + h * HD, 0 * D + h * HD + HD) for h in heads])
        kcols = np.concatenate(
            [np.arange(1 * D + h * HD, 1 * D + h * HD + HD) for h in heads])
        vcols = np.concatenate(
            [np.arange(2 * D + h * HD, 2 * D + h * HD + HD) for h in heads])
        cols = np.concatenate([qcols, kcols, vcols])
        wqkv = Wqkv[:, cols].copy()
        bq = bqkv[cols].copy()
        wqkv[:, :512] *= 0.125  # fold 1/sqrt(HD) into Q
        bq[:512] *= 0.125

        qkb = bq[:1024].reshape(8, 128).T.copy()       # [128, m-tile]
        bvr = np.broadcast_to(bq[1024:], (128, 512)).copy()

        expb = np.zeros((128, max(NBIAS, 1)), dtype=np.float32)
        for (j, mkey), col in BIAS_COLS.items():
            expb[:, col:col + 1] = SLOPES[2 * j + hs] * (p_idx - mkey)

        rows = np.concatenate(
            [np.arange(h * HD, h * HD + HD) for h in heads])
        wo = Wo[rows, :].copy()

        # pre-tile QK/V weights to match the SBUF tile layout so each load
        # is one contiguous [128, N] DMA:
        #   wqkvT[p, m, d, c] = wqkv[128d+p, 128m+c]   (m-major, 8x8x128)
        #   wvT[p, d, c]      = wqkv[128d+p, 1024+c]
        wqk = wqkv[:, :1024].reshape(8, 128, 8, 128)
        wqkvT = np.ascontiguousarray(
            wqk.transpose(1, 2, 0, 3).reshape(128, 8192)).astype(bf)
        wvp = wqkv[:, 1024:].reshape(8, 128, 512)
        wvT = np.ascontiguousarray(
            wvp.transpose(1, 0, 2).reshape(128, 4096)).astype(bf)

        xb = np.ascontiguousarray(x[b].T).astype(bf)
        xtq = np.ascontiguousarray(
            xb.reshape(8, 128, 4, 512).transpose(1, 2, 0, 3)
            .reshape(128, 16384))
        in_maps.append({
            "xTq": xtq,
            "wqkvT": wqkvT,
            "wvT": wvT,
            "wo": np.ascontiguousarray(wo).astype(bf),
            "qkb": np.ascontiguousarray(qkb),
            "bvr": bvr,
            "expb": expb,
            "tril": tril,
            "ones64": ones64,
            "vones": vones,
        })
    return in_maps


def kernel(x, mask, Wqkv, bqkv, Wo, bo, _trace=False):
    from concourse.bass_utils import run_bass_kernel_spmd

    nc = build_program()
    in_maps = make_inputs(x, mask, Wqkv, bqkv, Wo, bo)
    res = run_bass_kernel_spmd(nc, in_maps, core_ids=list(range(NCORES)),
                               trace=_trace, trace_cores=[0] if _trace else None)
    bo = np.asarray(bo, dtype=np.float32)
    out = np.empty((B, S, D), dtype=np.float32)
    for b in range(B):
        out[b] = res.results[b]["out_p"] + res.results[b + 4]["out_p"] + bo
    if _trace:
        kernel._last_result = res
    return out

